# revision 60
# baseline (speedup 1.0000x reference)
"""Single-head causal attention (B=8, T=2048, E=1024, H=64) on 8 trn2
cores, data-parallel over batch (one batch element per core).

Host-side prep (inside kernel(), before the device runs): x is cast to
bf16 (identical rounding to the on-chip cast it replaces) and the
projection weights are packed into the exact lhsT layouts the PE wants
(wqk[p, 128j+c] = [Wq|Wk][128j+p, c] bf16, wvb likewise).

Per-core pipeline:
  x bf16 --xbar DMA transpose, straight from DRAM--> xtg (chunk-major
      [p_e, (c, j, t128)] slabs; no staging tiles, no PE transposes,
      no drain copies)
  q/k: psum[128,512] = [Wq|Wk]^T @ xT -> qTt/kTt bf16 [64, T]
  v:   [128t, 64] = xT_chunk^T @ Wv (natural layout) -> v1 (ones col 64)
  scoresT[s-chunk, t] = kTt_j^T @ qTt  (bf16, diagonal-trimmed)
  wT = exp(scores/8) on ACT (psum -> sbuf bf16), diag tri-masked on DVE
  out_psum[128t, 65] += wT_j^T @ v1_j  (col 64 = softmax denominator)
  out = psum[:, 0:64] * 1/psum[:, 64] on DVE -> staged -> DMA out

Hard-won scheduling facts (all established by HW traces):
  - ALL DMAs ride the sync ring in strict FIFO order (group-0
    transposes, weights, remaining transposes, outputs - the ring opens
    with t0 at ~7us; weights still land before the first projection).  Two concurrent xbar transposes on
    different rings CORRUPT each other, and the xbar guard serializes
    transposes against every other in-flight DMA, so cross-ring
    traffic ping-pongs the stream.  An xbar transpose destination must
    be CONTIGUOUS per partition (strided dst = wrong data on HW).
  - The scalar (ACT) queue carries zero DMA instructions: DMA instrs
    carry Tile DMAHW-lane waits that head-of-line-block the exp
    ACTIVATEs behind them (exp is ACT-only and is the back-half
    critical path).  SWDGE (gpsimd) measures only ~60-130 GB/s here;
    one HWDGE ring ~183 GB/s; two rings ~330 (they share the 16 SDMA
    engines).
  - Loop order: scores/AV first (they feed ACT), arrival-gated ingest
    of the next group afterwards, so transposes never head-of-line-
    block score MMs in the PE FIFO.  Within ingest, qk before v: qk
    feeds scores->exp next loop, v is not consumed until AV a loop
    later.  Output chunks normalize eagerly as their AV chain ends.  Score psum slots are DEDICATED
    (3x 1-bank singles; sharing with the projection psum couples
    scores to the arrival-paced projection drains).
  - Tail (g=3, after ingest pools close): scores pair up - two full
    512-col score MMs into a [128,1024] 2-bank tile drained by ONE exp
    (2 pair slots; saves the ~293ns fixed ACTIVATE cost where ACT is
    critical).  wTp pairs j=2m/2m+1 per 512-col group block so a
    paired exp writes one contiguous [128,1024] range.
  - PSUM: 8 banks = pQK 1 + pV 1 + pAV 1 + pS 3 singles (+2 spare);
    tail swaps pQK/pV/spare for two [128,1024] pair slots.  start=True
    clears has_written for the WHOLE bank: one start per bank
    lifetime; AV accumulation chains stay i-major.
  - HAM: only bf16 matmuls register as PE-busy; a short bf16 dummy
    chain + 1-element exp pre-warm the clock and pre-load the ACT exp
    table.  Long filler chains measurably hurt (the kernel is not
    PE-clock-bound).  Run-to-run variance from HAM phase is several us.
"""

from contextlib import ExitStack

import numpy as np

import concourse.bass as bass
import concourse.mybir as mybir
from concourse.tile import TileContext, add_dep_helper
from concourse.masks import make_upper_triangular
from concourse.bass_utils import run_bass_kernel_spmd

B, T, E, H = 8, 2048, 1024, 64
NT = T // 128   # 16 t-chunks
NE = E // 128   # 8 e-chunks
NG = 4          # t-groups of 4 chunks / 512 cols
F32 = mybir.dt.float32
F32R = mybir.dt.float32r
BF16 = mybir.dt.bfloat16
SCALE = float(H) ** -0.5
EXP = mybir.ActivationFunctionType.Exp


def _split_excess_waits(nc: bass.Bass, cap: int = 1) -> int:
    n_split = 0
    for f in nc.m.functions:
        for bb in f.blocks:
            insts = list(bb.instructions)
            out = []
            dirty = False
            for inst in insts:
                si = inst.sync_info
                waits = list(si.on_wait) if si and si.on_wait else []
                if len(waits) > cap:
                    si.on_wait = waits[:cap]
                    for w in waits[cap:]:
                        nop = mybir.InstNoOp(
                            name=f"I-waitsplit-{n_split}", ins=[], outs=[]
                        )
                        nop.engine = inst.engine
                        nop.sync_info = mybir.SyncInfo(on_wait=[w], on_update=[])
                        out.append(nop)
                        n_split += 1
                    dirty = True
                out.append(inst)
            if dirty:
                bb.instructions = out
    return n_split


def build_nc(split_waits: bool = True) -> bass.Bass:
    nc = bass.Bass()
    x = nc.dram_tensor("x", [T, E], BF16, kind="ExternalInput")
    wqk_d = nc.dram_tensor("Wqk", [128, NE * 128], BF16, kind="ExternalInput")
    wvb_d = nc.dram_tensor("Wvb", [128, NE * H], BF16, kind="ExternalInput")
    out = nc.dram_tensor("out", [T, H], F32, kind="ExternalOutput")
    x_ap, out_ap = x.ap(), out.ap()

    with TileContext(nc) as tc:
        with (
            tc.tile_pool(name="const", bufs=1) as cpool,
            tc.tile_pool(name="wts", bufs=1) as wpool,
            tc.tile_pool(name="xtg", bufs=2) as xtpool,
            tc.tile_pool(name="qkv", bufs=1) as qkvpool,
            tc.tile_pool(name="wTp", bufs=8) as wtpool,
            tc.tile_pool(name="fin", bufs=8) as finpool,
            tc.tile_pool(name="pS", bufs=3, space="PSUM") as pS,
            tc.tile_pool(name="pAV", bufs=1, space="PSUM") as pAV,
        ):
            ctx_stack = ExitStack()
            ingest_pools = ExitStack()
            pQK = ingest_pools.enter_context(
                tc.tile_pool(name="pQK", bufs=1, space="PSUM")
            )
            pV = ingest_pools.enter_context(
                tc.tile_pool(name="pV", bufs=1, space="PSUM")
            )
            # ---- PE warm-up + ACT exp-table preload ----
            wdum = cpool.tile([128, 512], BF16, tag="wdum")
            nc.gpsimd.iota(wdum[:], pattern=[[1, 512]], channel_multiplier=7,
                           allow_small_or_imprecise_dtypes=True)
            tabscr = cpool.tile([128, 8], BF16, tag="tabscr")
            nc.vector.memset(tabscr[:], 0.0)
            nc.scalar.activation(tabscr[:, 0:1], tabscr[:, 4:5], EXP,
                                 scale=1.0)
            psd = pS.tile([128, 512], F32, tag="ps", name="psdummy")
            for k in range(11):
                nc.tensor.matmul(psd[:, 0:512], wdum[:, 0:128], wdum[:],
                                 start=True, stop=True)
            # ---- x + weights on the two HWDGE rings.  Group-0 chunks
            # first as half-chunks striped across both rings; weights
            # follow; the rest full-chunk DMAs alternating rings. ----
            # x rides the SWDGE (gpsimd) queue with an in-flight
            # f32->bf16 cast: (a) bf16 transposes run ~3x faster than the
            # fp32-HIGH path on the PE, (b) SWDGE is a third descriptor
            # path, so no DMA instruction ever head-of-line-blocks the
            # sync/scalar compute queues, (c) SBUF writes halve.  Tiles
            # are fully resident (no pool-WAR waits on any x DMA).
            # Group-0 chunks go individually for a fast start; later
            # groups as 2 MB quad-chunk DMAs for line-rate.
            # constants first: gpsimd ops must not queue behind DMAs
            tri = cpool.tile([128, 128], BF16, tag="tri")
            make_upper_triangular(nc, tri[:], val=1.0, diag=True)
            # ALL DMAs ride the sync ring in strict FIFO order (weights,
            # then the 16 chunk transposes, then outputs): the xbar
            # transpose guard serializes transposes against every other
            # in-flight DMA, so cross-ring traffic ping-pongs and
            # strangles the stream.  Weights arrive pre-packed bf16 from
            # the host (no on-chip prep).  The scalar (ACT) queue carries
            # zero DMA instructions.
            wqk = wpool.tile([128, NE * 128], BF16, tag="wqk")
            wvb = wpool.tile([128, NE * H], BF16, tag="wvb")

            def weight_dmas():
                # emitted AFTER the first four chunk transposes: the ring
                # opens with t0 ~2us earlier, and the weights still land
                # well before the first v/qk matmuls need them
                nc.sync.dma_start(wqk[:], wqk_d.ap())
                nc.sync.dma_start(wvb[:], wvb_d.ap())

            # ---- persistent tiles ----
            qTt = qkvpool.tile([64, T], BF16, tag="qTt")
            kTt = qkvpool.tile([64, T], BF16, tag="kTt")
            v1 = qkvpool.tile([128, NT * 65], BF16, tag="v1")
            nc.vector.memset(
                v1[:].rearrange("p (i c) -> p i c", c=65)[:, :, 64:65], 1.0
            )
            xtg = [
                xtpool.tile([128, NE * 512], BF16, tag="xtg", name=f"xtg{g}")
                for g in range(NG)
            ]
            # paired wT storage: wTp[m] lane0 = j=2m, lane1 = j=2m+1,
            # interleaved per 512-col t-group block
            wTp = [
                wtpool.tile([128, 2 * T], BF16, tag="wTp", name=f"wTp{m}")
                for m in range(NT // 2)
            ]

            def wT_ap(j, lo, hi):
                """Columns [lo:hi) (absolute t) of wT[j] in paired layout.
                Must not cross a 512-col group boundary."""
                g = lo // 512
                l0, l1 = lo - 512 * g, hi - 512 * g
                base = 1024 * g + 512 * (j % 2)
                return wTp[j // 2][:, base + l0 : base + l1]

            stage = [
                finpool.tile([128, 256], F32, tag="stage", bufs=2, name=f"st{g}")
                for g in range(NG)
            ]

            # ---------- emission unit generators ----------
            def t_units(g):
                """Transpose the 4 x-chunks of group g into xtg[g] via the
                DMA xbar, straight from DRAM (contiguous chunk-major
                destination slabs: a strided dst corrupts on HW)."""
                for c in range(4):
                    i = 4 * g + c
                    def unit(i=i, c=c):
                        dst = (
                            xtg[i // 4][:, 1024 * c : 1024 * c + 1024]
                            .rearrange("p (j t) -> p j t", t=128)
                        )
                        nc.sync.dma_start_transpose(
                            dst, x_ap[128 * i : 128 * i + 128, :]
                        )
                    yield unit

            def v_units(g):
                """v-projection for group g into the hi bank of the group's
                pQKV pair tile; unit c needs only chunk c's columns."""
                def v_unit(c):
                    def unit():
                        if c == 0:
                            v_units.pv = pV.tile([128, 512], F32, tag="pv",
                                                 name=f"pv{g}")
                        pv = v_units.pv
                        for j in range(NE):
                            nc.tensor.matmul(
                                pv[:, 64 * c : 64 * c + 64],
                                xtg[g][:, 1024 * c + 128 * j : 1024 * c + 128 * j + 128],
                                wvb[:, 64 * j : 64 * j + 64],
                                start=(j == 0),
                                stop=(j == NE - 1),
                            )
                        if c == 3:
                            i0 = 4 * g
                            dst = (
                                v1[:]
                                .rearrange("p (i c) -> p i c", c=65)[
                                    :, i0 : i0 + 4, 0:64
                                ]
                            )
                            nc.vector.tensor_copy(
                                dst,
                                v_units.pv[:, 0:256].rearrange(
                                    "p (i c) -> p i c", c=64
                                ),
                            )
                    return unit
                for c in range(4):
                    yield v_unit(c)

            def qk_units(g):
                """qk-projection for group g into the lo bank of the
                group's pQKV tile (needs the whole group transposed)."""
                def qk_unit(jpair):
                    def unit():
                        if jpair == 0:
                            qk_units.pqk = pQK.tile(
                                [128, 512], F32, tag="pqk", name=f"pqk{g}"
                            )
                        pqk = qk_units.pqk[:]
                        xv = xtg[g][:].rearrange(
                            "p (c j t) -> p j c t", c=4, t=128
                        )
                        for j in (2 * jpair, 2 * jpair + 1):
                            nc.tensor.matmul(
                                pqk[:].rearrange("p (c t) -> p c t", t=128),
                                wqk[:, 128 * j : 128 * j + 128],
                                xv[:, j],
                                start=(j == 0),
                                stop=(j == NE - 1),
                            )
                        if jpair == 3:
                            nc.vector.tensor_copy(
                                qTt[:, 512 * g : 512 * g + 512], pqk[0:64, :]
                            )
                            nc.vector.tensor_copy(
                                kTt[:, 512 * g : 512 * g + 512], pqk[64:128, :]
                            )
                    return unit
                for jp in range(4):
                    yield qk_unit(jp)

            def qkvn_units(g):
                # qk first: it feeds scores(g) -> exp (the ACT critical
                # path); v(g) is not consumed until AV(g) a full loop later
                yield from qk_units(g)
                yield from v_units(g)

            def next_slot():
                s_units.idx = (s_units.idx + 1) % len(s_units.slots)
                return s_units.slots[s_units.idx]

            def next_pair_slot():
                s_units.pidx = (s_units.pidx + 1) % len(s_units.pair_slots)
                return s_units.pair_slots[s_units.pidx]

            def s_unit_single(j, g):
                """One 512-col score MM + exp for s-chunk j over t-block g
                (trim + tri-mask when diagonal)."""
                def unit():
                    pool, tag = next_slot()
                    off = max(0, 128 * j - 512 * g)
                    ps = pool.tile([128, 512], F32, tag=tag,
                                   name=f"pss{g}_{j}")
                    nc.tensor.matmul(
                        ps[:, off:512],
                        kTt[:, 128 * j : 128 * j + 128],
                        qTt[:, 512 * g + off : 512 * g + 512],
                        start=True,
                        stop=True,
                    )
                    nc.scalar.activation(
                        wT_ap(j, 512 * g + off, 512 * g + 512),
                        ps[:, off:512],
                        EXP,
                        scale=SCALE,
                    )
                    if j >= 4 * g:
                        nc.vector.tensor_mul(
                            wT_ap(j, 128 * j, 128 * j + 128),
                            wT_ap(j, 128 * j, 128 * j + 128),
                            tri[:],
                        )
                return unit

            def s_unit_pair(m, g):
                """Tail-only: two full score MMs into a [128,1024] pair
                tile, drained by ONE exp (halves the ACT fixed cost where
                ACT is the critical path)."""
                def unit():
                    pool, tag = next_pair_slot()
                    ps = pool.tile([128, 1024], F32, tag=tag,
                                   name=f"psp{g}_{m}")
                    for half in range(2):
                        j = 2 * m + half
                        nc.tensor.matmul(
                            ps[:, 512 * half : 512 * half + 512],
                            kTt[:, 128 * j : 128 * j + 128],
                            qTt[:, 512 * g : 512 * g + 512],
                            start=True,
                            stop=True,
                        )
                    nc.scalar.activation(
                        wTp[m][:, 1024 * g : 1024 * g + 1024],
                        ps[:],
                        EXP,
                        scale=SCALE,
                    )
                return unit

            def s_units(g):
                if s_units.pair_slots is not None:
                    for m in range(2 * g):
                        yield s_unit_pair(m, g)
                    for c in range(4):
                        yield s_unit_single(4 * g + c, g)
                else:
                    for j in range(4 * g + 4):
                        yield s_unit_single(j, g)

            def av_units(g):
                """AV accumulation for the 4 t-chunks of group g (i-major
                chains; dual psum banks in the tail)."""
                dual = len(av_units.pools) == 2
                if dual:
                    def alloc(g=g):
                        (pa, ta), (pb, tb) = av_units.pools
                        av_units.tiles[g] = (
                            pa.tile([128, 130], F32, tag=ta, name=f"pava{g}"),
                            pb.tile([128, 130], F32, tag=tb, name=f"pavb{g}"),
                        )
                    yield alloc
                    for p in range(2):
                        c0, c1 = 2 * p, 2 * p + 1
                        i0, i1 = 4 * g + c0, 4 * g + c1
                        js = list(range(i1 + 1))
                        batches = [js[k : k + 4] for k in range(0, len(js), 4)]
                        for batch in batches:
                            def unit(p=p, i0=i0, i1=i1, batch=batch, g=g):
                                pava, pavb = av_units.tiles[g]
                                off = 65 * p
                                for j in batch:
                                    if j <= i0:
                                        nc.tensor.matmul(
                                            pava[:, off : off + 65],
                                            wT_ap(j, 128 * i0, 128 * i0 + 128),
                                            v1[:, 65 * j : 65 * j + 65],
                                            start=(j == 0),
                                            stop=(j == i0),
                                        )
                                    nc.tensor.matmul(
                                        pavb[:, off : off + 65],
                                        wT_ap(j, 128 * i1, 128 * i1 + 128),
                                        v1[:, 65 * j : 65 * j + 65],
                                        start=(j == 0),
                                        stop=(j == i1),
                                    )
                            yield unit
                        # normalize + ship this chunk-pair immediately so
                        # the final output DMA covers only the last pair
                        def norm_out(p=p, c0=c0, c1=c1, g=g):
                            pava, pavb = av_units.tiles[g]
                            off = 65 * p
                            for pv_, c_ in ((pava, c0), (pavb, c1)):
                                rcp = finpool.tile([128, 1], F32, tag="rcp",
                                                   bufs=4)
                                nc.vector.reciprocal(
                                    rcp[:], pv_[:, off + 64 : off + 65]
                                )
                                nc.vector.tensor_scalar_mul(
                                    stage[g][:, 64 * c_ : 64 * c_ + 64],
                                    pv_[:, off : off + 64],
                                    rcp[:],
                                )
                            nc.sync.dma_start(
                                out_ap[512 * g + 256 * p : 512 * g + 256 * p + 256, :]
                                .rearrange("(c p2) h -> p2 c h", p2=128),
                                stage[g][:, 128 * p : 128 * p + 128]
                                .rearrange("p (c h) -> p c h", h=64),
                            )
                        yield norm_out
                else:
                    def alloc(g=g):
                        pa, ta = av_units.pools[0]
                        av_units.tiles[g] = pa.tile(
                            [128, 260], F32, tag=ta, name=f"pav{g}"
                        )
                    yield alloc
                    for c in range(4):
                        i = 4 * g + c
                        js = list(range(i + 1))
                        batches = [js[k : k + 4] for k in range(0, len(js), 4)]
                        for bi, batch in enumerate(batches):
                            def unit(i=i, c=c, batch=batch, g=g,
                                     last=(bi == len(batches) - 1)):
                                pav = av_units.tiles[g]
                                for j in batch:
                                    nc.tensor.matmul(
                                        pav[:, 65 * c : 65 * c + 65],
                                        wT_ap(j, 128 * i, 128 * i + 128),
                                        v1[:, 65 * j : 65 * j + 65],
                                        start=(j == 0),
                                        stop=(j == i),
                                    )
                                if last:
                                    # normalize this chunk as soon as its
                                    # chain completes (shortens the tail)
                                    rcp = finpool.tile([128, 1], F32,
                                                       tag="rcp", bufs=4)
                                    nc.vector.reciprocal(
                                        rcp[:],
                                        pav[:, 65 * c + 64 : 65 * c + 65],
                                    )
                                    nc.vector.tensor_scalar_mul(
                                        stage[g][:, 64 * c : 64 * c + 64],
                                        pav[:, 65 * c : 65 * c + 64],
                                        rcp[:],
                                    )
                            yield unit
                if not dual:
                    def dma_out_a(g=g):
                        nc.sync.dma_start(
                            out_ap[512 * g : 512 * g + 256, :]
                            .rearrange("(c p) h -> p c h", p=128),
                            stage[g][:, 0:128]
                            .rearrange("p (c h) -> p c h", h=64),
                        )
                    def dma_out_b(g=g):
                        nc.sync.dma_start(
                            out_ap[512 * g + 256 : 512 * g + 512, :]
                            .rearrange("(c p) h -> p c h", p=128),
                            stage[g][:, 128:256]
                            .rearrange("p (c h) -> p c h", h=64),
                        )
                    yield dma_out_a
                    yield dma_out_b

            def drain(*streams):
                streams = [s for s in streams if s is not None]
                while streams:
                    nxt = []
                    for s in streams:
                        u = next(s, None)
                        if u is not None:
                            u()
                            nxt.append(s)
                    streams = nxt

            s_units.slots = [(pS, "ps")]
            s_units.idx = 0
            s_units.pair_slots = None
            av_units.pools = [(pAV, "pav")]
            av_units.tiles = {}

            # ---------- prologue: group 0, v-proj fills chunk gaps ----------
            tu = list(t_units(0))
            for c in range(4):
                tu[c]()
            weight_dmas()
            drain(qk_units(0))
            drain(v_units(0))

            # ---------- steady loop ----------
            for g in range(NG):
                if g == NG - 1:
                    # transposes/projections done: free their psum banks and
                    # widen the scores rotation for the exp-heavy last group
                    ingest_pools.close()
                    pS3 = ctx_stack.enter_context(
                        tc.tile_pool(name="pS3", bufs=1, space="PSUM")
                    )
                    pS4 = ctx_stack.enter_context(
                        tc.tile_pool(name="pS4", bufs=1, space="PSUM")
                    )
                    s_units.pair_slots = [(pS3, "ps3"), (pS4, "ps4")]
                    s_units.pidx = 0
                # scores/AV first (they feed ACT, the back-half critical
                # resource); the arrival-gated ingest of g+1 afterwards so
                # its transposes never head-of-line-block score MMs.
                a = s_units(g)
                b = av_units(g - 1) if g >= 1 else None
                drain(a, b)
                if g + 1 < NG:
                    drain(t_units(g + 1))
                    drain(qkvn_units(g + 1))

            # ---------- epilogue: AV of the last group ----------
            drain(av_units(NG - 1))
            ctx_stack.close()

    if split_waits:
        _split_excess_waits(nc)
    return nc


_NC_CACHE = None


def _get_nc() -> bass.Bass:
    global _NC_CACHE
    if _NC_CACHE is None:
        _NC_CACHE = build_nc()
    return _NC_CACHE


def kernel(x, Wq, Wk, Wv, **run_kwargs):
    import ml_dtypes
    nc = _get_nc()
    x = np.ascontiguousarray(x).astype(ml_dtypes.bfloat16)
    # pre-pack the projection weights on the host exactly as the PE wants
    # them: wqk[p, 128j+c] = [Wq|Wk][128j+p, c], wvb[p, 64j+h] = Wv[128j+p, h]
    wq_r = Wq.reshape(NE, 128, H)
    wk_r = Wk.reshape(NE, 128, H)
    wv_r = Wv.reshape(NE, 128, H)
    wqk_np = np.concatenate([wq_r, wk_r], axis=2)      # [NE, 128, 128]
    wqk_np = wqk_np.transpose(1, 0, 2).reshape(128, NE * 128)
    wvb_np = wv_r.transpose(1, 0, 2).reshape(128, NE * H)
    wqk_np = np.ascontiguousarray(wqk_np).astype(ml_dtypes.bfloat16)
    wvb_np = np.ascontiguousarray(wvb_np).astype(ml_dtypes.bfloat16)
    in_maps = [
        {
            "x": np.ascontiguousarray(x[b]),
            "Wqk": wqk_np,
            "Wvb": wvb_np,
        }
        for b in range(B)
    ]
    res = run_bass_kernel_spmd(nc, in_maps, core_ids=list(range(B)), **run_kwargs)
    out = np.stack([res.results[b]["out"] for b in range(B)], axis=0)
    kernel.last_results = res
    return out


# revision 61
# speedup vs baseline: 1.0325x; 1.0325x over previous
"""Single-head causal attention (B=8, T=2048, E=1024, H=64) on 8 trn2
cores, data-parallel over batch (one batch element per core).

Host-side prep (inside kernel(), before the device runs): x is cast to
bf16 (identical rounding to the on-chip cast it replaces) and the
projection weights are packed into the exact lhsT layouts the PE wants
(wqk[p, 128j+c] = [Wq|Wk][128j+p, c] bf16, wvb likewise).

Per-core pipeline:
  x bf16 --xbar DMA transpose, straight from DRAM--> xtg (chunk-major
      [p_e, (c, j, t128)] slabs; no staging tiles, no PE transposes,
      no drain copies)
  q/k: psum[128,512] = [Wq|Wk]^T @ xT -> qTt/kTt bf16 [64, T]
  v:   [128t, 64] = xT_chunk^T @ Wv (natural layout) -> v1 (ones col 64)
  scoresT[s-chunk, t] = kTt_j^T @ qTt  (bf16, diagonal-trimmed)
  wT = exp(scores/8) on ACT (psum -> sbuf bf16), diag tri-masked on DVE
  out_psum[128t, 65] += wT_j^T @ v1_j  (col 64 = softmax denominator)
  out = psum[:, 0:64] * 1/psum[:, 64] on DVE -> staged -> DMA out

Hard-won scheduling facts (all established by HW traces):
  - ALL DMAs ride the sync ring in strict FIFO order (group-0
    transposes, weights, remaining transposes, outputs - the ring opens
    with t0 at ~7us; weights still land before the first projection).  Two concurrent xbar transposes on
    different rings CORRUPT each other, and the xbar guard serializes
    transposes against every other in-flight DMA, so cross-ring
    traffic ping-pongs the stream.  An xbar transpose destination must
    be CONTIGUOUS per partition (strided dst = wrong data on HW).
  - The scalar (ACT) queue carries zero DMA instructions: DMA instrs
    carry Tile DMAHW-lane waits that head-of-line-block the exp
    ACTIVATEs behind them (exp is ACT-only and is the back-half
    critical path).  SWDGE (gpsimd) measures only ~60-130 GB/s here;
    one HWDGE ring ~183 GB/s; two rings ~330 (they share the 16 SDMA
    engines).
  - Loop order: scores/AV first (they feed ACT), arrival-gated ingest
    of the next group afterwards, so transposes never head-of-line-
    block score MMs in the PE FIFO.  Within ingest, qk before v: qk
    feeds scores->exp next loop, v is not consumed until AV a loop
    later.  Output chunks normalize eagerly as their AV chain ends.  Score psum slots are DEDICATED
    (3x 1-bank singles; sharing with the projection psum couples
    scores to the arrival-paced projection drains).
  - Tail (g=3, after ingest pools close): scores pair up - two full
    512-col score MMs into a [128,1024] 2-bank tile drained by ONE exp
    (2 pair slots; saves the ~293ns fixed ACTIVATE cost where ACT is
    critical).  wTp pairs j=2m/2m+1 per 512-col group block so a
    paired exp writes one contiguous [128,1024] range.
  - PSUM: 8 banks = pQK 1 + pV 1 + pAV 1 + pS 3 singles (+2 spare);
    tail swaps pQK/pV/spare for two [128,1024] pair slots.  start=True
    clears has_written for the WHOLE bank: one start per bank
    lifetime; AV accumulation chains stay i-major.
  - HAM: only bf16 matmuls register as PE-busy; a short bf16 dummy
    chain + 1-element exp pre-warm the clock and pre-load the ACT exp
    table.  Long filler chains measurably hurt (the kernel is not
    PE-clock-bound).  Run-to-run variance from HAM phase is several us.
"""

from contextlib import ExitStack

import numpy as np

import concourse.bass as bass
import concourse.mybir as mybir
from concourse.tile import TileContext, add_dep_helper
from concourse.masks import make_upper_triangular
from concourse.bass_utils import run_bass_kernel_spmd

B, T, E, H = 8, 2048, 1024, 64
NT = T // 128   # 16 t-chunks
NE = E // 128   # 8 e-chunks
NG = 4          # t-groups of 4 chunks / 512 cols
F32 = mybir.dt.float32
F32R = mybir.dt.float32r
BF16 = mybir.dt.bfloat16
SCALE = float(H) ** -0.5
EXP = mybir.ActivationFunctionType.Exp


def _split_excess_waits(nc: bass.Bass, cap: int = 1) -> int:
    n_split = 0
    for f in nc.m.functions:
        for bb in f.blocks:
            insts = list(bb.instructions)
            out = []
            dirty = False
            for inst in insts:
                si = inst.sync_info
                waits = list(si.on_wait) if si and si.on_wait else []
                if len(waits) > cap:
                    si.on_wait = waits[:cap]
                    for w in waits[cap:]:
                        nop = mybir.InstNoOp(
                            name=f"I-waitsplit-{n_split}", ins=[], outs=[]
                        )
                        nop.engine = inst.engine
                        nop.sync_info = mybir.SyncInfo(on_wait=[w], on_update=[])
                        out.append(nop)
                        n_split += 1
                    dirty = True
                out.append(inst)
            if dirty:
                bb.instructions = out
    return n_split


def build_nc(split_waits: bool = True) -> bass.Bass:
    nc = bass.Bass()
    x = nc.dram_tensor("x", [T, E], BF16, kind="ExternalInput")
    wqk_d = nc.dram_tensor("Wqk", [128, NE * 128], BF16, kind="ExternalInput")
    wvb_d = nc.dram_tensor("Wvb", [128, NE * H], BF16, kind="ExternalInput")
    out = nc.dram_tensor("out", [T, H], F32, kind="ExternalOutput")
    x_ap, out_ap = x.ap(), out.ap()

    with TileContext(nc) as tc:
        with (
            tc.tile_pool(name="const", bufs=1) as cpool,
            tc.tile_pool(name="wts", bufs=1) as wpool,
            tc.tile_pool(name="xtg", bufs=4) as xtpool,
            tc.tile_pool(name="qkv", bufs=1) as qkvpool,
            tc.tile_pool(name="wTp", bufs=8) as wtpool,
            tc.tile_pool(name="fin", bufs=8) as finpool,
            tc.tile_pool(name="pS", bufs=3, space="PSUM") as pS,
            tc.tile_pool(name="pAV", bufs=1, space="PSUM") as pAV,
        ):
            ctx_stack = ExitStack()
            ingest_pools = ExitStack()
            pQK = ingest_pools.enter_context(
                tc.tile_pool(name="pQK", bufs=1, space="PSUM")
            )
            pV = ingest_pools.enter_context(
                tc.tile_pool(name="pV", bufs=1, space="PSUM")
            )
            # ---- PE warm-up + ACT exp-table preload ----
            wdum = cpool.tile([128, 512], BF16, tag="wdum")
            nc.gpsimd.iota(wdum[:], pattern=[[1, 512]], channel_multiplier=7,
                           allow_small_or_imprecise_dtypes=True)
            tabscr = cpool.tile([128, 8], BF16, tag="tabscr")
            nc.vector.memset(tabscr[:], 0.0)
            nc.scalar.activation(tabscr[:, 0:1], tabscr[:, 4:5], EXP,
                                 scale=1.0)
            psd = pS.tile([128, 512], F32, tag="ps", name="psdummy")
            for k in range(11):
                nc.tensor.matmul(psd[:, 0:512], wdum[:, 0:128], wdum[:],
                                 start=True, stop=True)
            # ---- x + weights on the two HWDGE rings.  Group-0 chunks
            # first as half-chunks striped across both rings; weights
            # follow; the rest full-chunk DMAs alternating rings. ----
            # x rides the SWDGE (gpsimd) queue with an in-flight
            # f32->bf16 cast: (a) bf16 transposes run ~3x faster than the
            # fp32-HIGH path on the PE, (b) SWDGE is a third descriptor
            # path, so no DMA instruction ever head-of-line-blocks the
            # sync/scalar compute queues, (c) SBUF writes halve.  Tiles
            # are fully resident (no pool-WAR waits on any x DMA).
            # Group-0 chunks go individually for a fast start; later
            # groups as 2 MB quad-chunk DMAs for line-rate.
            # constants first: gpsimd ops must not queue behind DMAs
            tri = cpool.tile([128, 128], BF16, tag="tri")
            make_upper_triangular(nc, tri[:], val=1.0, diag=True)
            # ALL DMAs ride the sync ring in strict FIFO order (weights,
            # then the 16 chunk transposes, then outputs): the xbar
            # transpose guard serializes transposes against every other
            # in-flight DMA, so cross-ring traffic ping-pongs and
            # strangles the stream.  Weights arrive pre-packed bf16 from
            # the host (no on-chip prep).  The scalar (ACT) queue carries
            # zero DMA instructions.
            wqk = wpool.tile([128, NE * 128], BF16, tag="wqk")
            wvb = wpool.tile([128, NE * H], BF16, tag="wvb")

            def weight_dmas():
                # emitted AFTER the first four chunk transposes: the ring
                # opens with t0 ~2us earlier, and the weights still land
                # well before the first v/qk matmuls need them
                nc.sync.dma_start(wqk[:], wqk_d.ap())
                nc.sync.dma_start(wvb[:], wvb_d.ap())

            # ---- persistent tiles ----
            qTt = qkvpool.tile([64, T], BF16, tag="qTt")
            kTt = qkvpool.tile([64, T], BF16, tag="kTt")
            v1 = qkvpool.tile([128, NT * 65], BF16, tag="v1")
            nc.vector.memset(
                v1[:].rearrange("p (i c) -> p i c", c=65)[:, :, 64:65], 1.0
            )
            xtg = [
                xtpool.tile([128, NE * 512], BF16, tag="xtg", name=f"xtg{g}")
                for g in range(NG)
            ]
            # paired wT storage: wTp[m] lane0 = j=2m, lane1 = j=2m+1,
            # interleaved per 512-col t-group block
            wTp = [
                wtpool.tile([128, 2 * T], BF16, tag="wTp", name=f"wTp{m}")
                for m in range(NT // 2)
            ]

            def wT_ap(j, lo, hi):
                """Columns [lo:hi) (absolute t) of wT[j] in paired layout.
                Must not cross a 512-col group boundary."""
                g = lo // 512
                l0, l1 = lo - 512 * g, hi - 512 * g
                base = 1024 * g + 512 * (j % 2)
                return wTp[j // 2][:, base + l0 : base + l1]

            stage = [
                finpool.tile([128, 256], F32, tag="stage", bufs=2, name=f"st{g}")
                for g in range(NG)
            ]

            # ---------- emission unit generators ----------
            def t_units(g):
                """Transpose the 4 x-chunks of group g into xtg[g] via the
                DMA xbar, straight from DRAM (contiguous chunk-major
                destination slabs: a strided dst corrupts on HW)."""
                for c in range(4):
                    i = 4 * g + c
                    def unit(i=i, c=c):
                        dst = (
                            xtg[i // 4][:, 1024 * c : 1024 * c + 1024]
                            .rearrange("p (j t) -> p j t", t=128)
                        )
                        nc.sync.dma_start_transpose(
                            dst, x_ap[128 * i : 128 * i + 128, :]
                        )
                    yield unit

            def v_units(g):
                """v-projection for group g into the hi bank of the group's
                pQKV pair tile; unit c needs only chunk c's columns."""
                def v_unit(c):
                    def unit():
                        if c == 0:
                            v_units.pv = pV.tile([128, 512], F32, tag="pv",
                                                 name=f"pv{g}")
                        pv = v_units.pv
                        for j in range(NE):
                            nc.tensor.matmul(
                                pv[:, 64 * c : 64 * c + 64],
                                xtg[g][:, 1024 * c + 128 * j : 1024 * c + 128 * j + 128],
                                wvb[:, 64 * j : 64 * j + 64],
                                start=(j == 0),
                                stop=(j == NE - 1),
                            )
                        if c == 3:
                            i0 = 4 * g
                            dst = (
                                v1[:]
                                .rearrange("p (i c) -> p i c", c=65)[
                                    :, i0 : i0 + 4, 0:64
                                ]
                            )
                            nc.vector.tensor_copy(
                                dst,
                                v_units.pv[:, 0:256].rearrange(
                                    "p (i c) -> p i c", c=64
                                ),
                            )
                    return unit
                for c in range(4):
                    yield v_unit(c)

            def qk_units(g):
                """qk-projection for group g into the lo bank of the
                group's pQKV tile (needs the whole group transposed)."""
                def qk_unit(jpair):
                    def unit():
                        if jpair == 0:
                            qk_units.pqk = pQK.tile(
                                [128, 512], F32, tag="pqk", name=f"pqk{g}"
                            )
                        pqk = qk_units.pqk[:]
                        xv = xtg[g][:].rearrange(
                            "p (c j t) -> p j c t", c=4, t=128
                        )
                        for j in (2 * jpair, 2 * jpair + 1):
                            nc.tensor.matmul(
                                pqk[:].rearrange("p (c t) -> p c t", t=128),
                                wqk[:, 128 * j : 128 * j + 128],
                                xv[:, j],
                                start=(j == 0),
                                stop=(j == NE - 1),
                            )
                        if jpair == 3:
                            nc.vector.tensor_copy(
                                qTt[:, 512 * g : 512 * g + 512], pqk[0:64, :]
                            )
                            nc.vector.tensor_copy(
                                kTt[:, 512 * g : 512 * g + 512], pqk[64:128, :]
                            )
                    return unit
                for jp in range(4):
                    yield qk_unit(jp)

            def qkvn_units(g):
                # qk first: it feeds scores(g) -> exp (the ACT critical
                # path); v(g) is not consumed until AV(g) a full loop later
                yield from qk_units(g)
                yield from v_units(g)

            def next_slot():
                s_units.idx = (s_units.idx + 1) % len(s_units.slots)
                return s_units.slots[s_units.idx]

            def next_pair_slot():
                s_units.pidx = (s_units.pidx + 1) % len(s_units.pair_slots)
                return s_units.pair_slots[s_units.pidx]

            def s_unit_single(j, g):
                """One 512-col score MM + exp for s-chunk j over t-block g
                (trim + tri-mask when diagonal)."""
                def unit():
                    pool, tag = next_slot()
                    off = max(0, 128 * j - 512 * g)
                    ps = pool.tile([128, 512], F32, tag=tag,
                                   name=f"pss{g}_{j}")
                    nc.tensor.matmul(
                        ps[:, off:512],
                        kTt[:, 128 * j : 128 * j + 128],
                        qTt[:, 512 * g + off : 512 * g + 512],
                        start=True,
                        stop=True,
                    )
                    nc.scalar.activation(
                        wT_ap(j, 512 * g + off, 512 * g + 512),
                        ps[:, off:512],
                        EXP,
                        scale=SCALE,
                    )
                    if j >= 4 * g:
                        nc.vector.tensor_mul(
                            wT_ap(j, 128 * j, 128 * j + 128),
                            wT_ap(j, 128 * j, 128 * j + 128),
                            tri[:],
                        )
                return unit

            def s_unit_pair(m, g):
                """Tail-only: two full score MMs into a [128,1024] pair
                tile, drained by ONE exp (halves the ACT fixed cost where
                ACT is the critical path)."""
                def unit():
                    pool, tag = next_pair_slot()
                    ps = pool.tile([128, 1024], F32, tag=tag,
                                   name=f"psp{g}_{m}")
                    for half in range(2):
                        j = 2 * m + half
                        nc.tensor.matmul(
                            ps[:, 512 * half : 512 * half + 512],
                            kTt[:, 128 * j : 128 * j + 128],
                            qTt[:, 512 * g : 512 * g + 512],
                            start=True,
                            stop=True,
                        )
                    nc.scalar.activation(
                        wTp[m][:, 1024 * g : 1024 * g + 1024],
                        ps[:],
                        EXP,
                        scale=SCALE,
                    )
                return unit

            def s_units(g):
                if s_units.pair_slots is not None:
                    for m in range(2 * g):
                        yield s_unit_pair(m, g)
                    for c in range(4):
                        yield s_unit_single(4 * g + c, g)
                else:
                    for j in range(4 * g + 4):
                        yield s_unit_single(j, g)

            def av_units(g):
                """AV accumulation for the 4 t-chunks of group g (i-major
                chains; dual psum banks in the tail)."""
                dual = len(av_units.pools) == 2
                if dual:
                    def alloc(g=g):
                        (pa, ta), (pb, tb) = av_units.pools
                        av_units.tiles[g] = (
                            pa.tile([128, 130], F32, tag=ta, name=f"pava{g}"),
                            pb.tile([128, 130], F32, tag=tb, name=f"pavb{g}"),
                        )
                    yield alloc
                    for p in range(2):
                        c0, c1 = 2 * p, 2 * p + 1
                        i0, i1 = 4 * g + c0, 4 * g + c1
                        js = list(range(i1 + 1))
                        batches = [js[k : k + 4] for k in range(0, len(js), 4)]
                        for batch in batches:
                            def unit(p=p, i0=i0, i1=i1, batch=batch, g=g):
                                pava, pavb = av_units.tiles[g]
                                off = 65 * p
                                for j in batch:
                                    if j <= i0:
                                        nc.tensor.matmul(
                                            pava[:, off : off + 65],
                                            wT_ap(j, 128 * i0, 128 * i0 + 128),
                                            v1[:, 65 * j : 65 * j + 65],
                                            start=(j == 0),
                                            stop=(j == i0),
                                        )
                                    nc.tensor.matmul(
                                        pavb[:, off : off + 65],
                                        wT_ap(j, 128 * i1, 128 * i1 + 128),
                                        v1[:, 65 * j : 65 * j + 65],
                                        start=(j == 0),
                                        stop=(j == i1),
                                    )
                            yield unit
                        # normalize + ship this chunk-pair immediately so
                        # the final output DMA covers only the last pair
                        def norm_out(p=p, c0=c0, c1=c1, g=g):
                            pava, pavb = av_units.tiles[g]
                            off = 65 * p
                            for pv_, c_ in ((pava, c0), (pavb, c1)):
                                rcp = finpool.tile([128, 1], F32, tag="rcp",
                                                   bufs=4)
                                nc.vector.reciprocal(
                                    rcp[:], pv_[:, off + 64 : off + 65]
                                )
                                nc.vector.tensor_scalar_mul(
                                    stage[g][:, 64 * c_ : 64 * c_ + 64],
                                    pv_[:, off : off + 64],
                                    rcp[:],
                                )
                            nc.sync.dma_start(
                                out_ap[512 * g + 256 * p : 512 * g + 256 * p + 256, :]
                                .rearrange("(c p2) h -> p2 c h", p2=128),
                                stage[g][:, 128 * p : 128 * p + 128]
                                .rearrange("p (c h) -> p c h", h=64),
                            )
                        yield norm_out
                else:
                    def alloc(g=g):
                        pa, ta = av_units.pools[0]
                        av_units.tiles[g] = pa.tile(
                            [128, 260], F32, tag=ta, name=f"pav{g}"
                        )
                    yield alloc
                    for c in range(4):
                        i = 4 * g + c
                        js = list(range(i + 1))
                        batches = [js[k : k + 4] for k in range(0, len(js), 4)]
                        for bi, batch in enumerate(batches):
                            def unit(i=i, c=c, batch=batch, g=g,
                                     last=(bi == len(batches) - 1)):
                                pav = av_units.tiles[g]
                                for j in batch:
                                    nc.tensor.matmul(
                                        pav[:, 65 * c : 65 * c + 65],
                                        wT_ap(j, 128 * i, 128 * i + 128),
                                        v1[:, 65 * j : 65 * j + 65],
                                        start=(j == 0),
                                        stop=(j == i),
                                    )
                                if last:
                                    # normalize this chunk as soon as its
                                    # chain completes (shortens the tail)
                                    rcp = finpool.tile([128, 1], F32,
                                                       tag="rcp", bufs=4)
                                    nc.vector.reciprocal(
                                        rcp[:],
                                        pav[:, 65 * c + 64 : 65 * c + 65],
                                    )
                                    nc.vector.tensor_scalar_mul(
                                        stage[g][:, 64 * c : 64 * c + 64],
                                        pav[:, 65 * c : 65 * c + 64],
                                        rcp[:],
                                    )
                            yield unit
                if not dual:
                    def dma_out_a(g=g):
                        nc.sync.dma_start(
                            out_ap[512 * g : 512 * g + 256, :]
                            .rearrange("(c p) h -> p c h", p=128),
                            stage[g][:, 0:128]
                            .rearrange("p (c h) -> p c h", h=64),
                        )
                    def dma_out_b(g=g):
                        nc.sync.dma_start(
                            out_ap[512 * g + 256 : 512 * g + 512, :]
                            .rearrange("(c p) h -> p c h", p=128),
                            stage[g][:, 128:256]
                            .rearrange("p (c h) -> p c h", h=64),
                        )
                    yield dma_out_a
                    yield dma_out_b

            def drain(*streams):
                streams = [s for s in streams if s is not None]
                while streams:
                    nxt = []
                    for s in streams:
                        u = next(s, None)
                        if u is not None:
                            u()
                            nxt.append(s)
                    streams = nxt

            s_units.slots = [(pS, "ps")]
            s_units.idx = 0
            s_units.pair_slots = None
            av_units.pools = [(pAV, "pav")]
            av_units.tiles = {}

            # ---------- prologue: group 0, v-proj fills chunk gaps ----------
            tu = list(t_units(0))
            for c in range(4):
                tu[c]()
            weight_dmas()
            drain(qk_units(0))
            drain(v_units(0))

            # ---------- steady loop ----------
            for g in range(NG):
                if g == NG - 1:
                    # transposes/projections done: free their psum banks and
                    # widen the scores rotation for the exp-heavy last group
                    ingest_pools.close()
                    pS3 = ctx_stack.enter_context(
                        tc.tile_pool(name="pS3", bufs=1, space="PSUM")
                    )
                    pS4 = ctx_stack.enter_context(
                        tc.tile_pool(name="pS4", bufs=1, space="PSUM")
                    )
                    s_units.pair_slots = [(pS3, "ps3"), (pS4, "ps4")]
                    s_units.pidx = 0
                # scores/AV first (they feed ACT, the back-half critical
                # resource); the arrival-gated ingest of g+1 afterwards so
                # its transposes never head-of-line-block score MMs.
                a = s_units(g)
                b = av_units(g - 1) if g >= 1 else None
                drain(a, b)
                if g + 1 < NG:
                    drain(t_units(g + 1))
                    drain(qkvn_units(g + 1))

            # ---------- epilogue: AV of the last group ----------
            drain(av_units(NG - 1))
            ctx_stack.close()

    if split_waits:
        _split_excess_waits(nc)
    return nc


_NC_CACHE = None


def _get_nc() -> bass.Bass:
    global _NC_CACHE
    if _NC_CACHE is None:
        _NC_CACHE = build_nc()
    return _NC_CACHE


def kernel(x, Wq, Wk, Wv, **run_kwargs):
    import ml_dtypes
    nc = _get_nc()
    x = np.ascontiguousarray(x).astype(ml_dtypes.bfloat16)
    # pre-pack the projection weights on the host exactly as the PE wants
    # them: wqk[p, 128j+c] = [Wq|Wk][128j+p, c], wvb[p, 64j+h] = Wv[128j+p, h]
    wq_r = Wq.reshape(NE, 128, H)
    wk_r = Wk.reshape(NE, 128, H)
    wv_r = Wv.reshape(NE, 128, H)
    wqk_np = np.concatenate([wq_r, wk_r], axis=2)      # [NE, 128, 128]
    wqk_np = wqk_np.transpose(1, 0, 2).reshape(128, NE * 128)
    wvb_np = wv_r.transpose(1, 0, 2).reshape(128, NE * H)
    wqk_np = np.ascontiguousarray(wqk_np).astype(ml_dtypes.bfloat16)
    wvb_np = np.ascontiguousarray(wvb_np).astype(ml_dtypes.bfloat16)
    in_maps = [
        {
            "x": np.ascontiguousarray(x[b]),
            "Wqk": wqk_np,
            "Wvb": wvb_np,
        }
        for b in range(B)
    ]
    res = run_bass_kernel_spmd(nc, in_maps, core_ids=list(range(B)), **run_kwargs)
    out = np.stack([res.results[b]["out"] for b in range(B)], axis=0)
    kernel.last_results = res
    return out


# revision 62
# speedup vs baseline: 1.2250x; 1.1864x over previous
"""Single-head causal attention (B=8, T=2048, E=1024, H=64) on 8 trn2
cores, data-parallel over batch (one batch element per core).

Host-side prep (inside kernel(), before the device runs): x is cast to
bf16 (identical rounding to the on-chip cast it replaces) and the
projection weights are packed into the exact lhsT layouts the PE wants
(wqk[p, 128j+c] = [Wq|Wk][128j+p, c] bf16, wvb likewise).

Per-core pipeline:
  x bf16 --xbar DMA transpose, straight from DRAM--> xtg (chunk-major
      [p_e, (c, j, t128)] slabs; no staging tiles, no PE transposes,
      no drain copies)
  q/k: psum[128,512] = [Wq|Wk]^T @ xT -> qTt/kTt bf16 [64, T]
  v:   [128t, 64] = xT_chunk^T @ Wv (natural layout) -> v1 (ones col 64)
  scoresT[s-chunk, t] = kTt_j^T @ qTt  (bf16, diagonal-trimmed)
  wT = exp(scores/8) on ACT (psum -> sbuf bf16), diag tri-masked on DVE
  out_psum[128t, 65] += wT_j^T @ v1_j  (col 64 = softmax denominator)
  out = psum[:, 0:64] * 1/psum[:, 64] on DVE -> staged -> DMA out

Hard-won scheduling facts (all established by HW traces):
  - ALL DMAs ride the sync ring in strict FIFO order (group-0
    transposes, weights, remaining transposes, outputs - the ring opens
    with t0 at ~7us; weights still land before the first projection).  Two concurrent xbar transposes on
    different rings CORRUPT each other, and the xbar guard serializes
    transposes against every other in-flight DMA, so cross-ring
    traffic ping-pongs the stream.  An xbar transpose destination must
    be CONTIGUOUS per partition (strided dst = wrong data on HW).
  - The scalar (ACT) queue carries zero DMA instructions: DMA instrs
    carry Tile DMAHW-lane waits that head-of-line-block the exp
    ACTIVATEs behind them (exp is ACT-only and is the back-half
    critical path).  SWDGE (gpsimd) measures only ~60-130 GB/s here;
    one HWDGE ring ~183 GB/s; two rings ~330 (they share the 16 SDMA
    engines).
  - Loop order: scores/AV first (they feed ACT), arrival-gated ingest
    of the next group afterwards, so transposes never head-of-line-
    block score MMs in the PE FIFO.  Within ingest, qk before v: qk
    feeds scores->exp next loop, v is not consumed until AV a loop
    later.  Output chunks normalize eagerly as their AV chain ends.  Score psum slots are DEDICATED
    (3x 1-bank singles; sharing with the projection psum couples
    scores to the arrival-paced projection drains).
  - Tail (g=3, after ingest pools close): scores pair up - two full
    512-col score MMs into a [128,1024] 2-bank tile drained by ONE exp
    (2 pair slots; saves the ~293ns fixed ACTIVATE cost where ACT is
    critical).  wTp pairs j=2m/2m+1 per 512-col group block so a
    paired exp writes one contiguous [128,1024] range.
  - PSUM: 8 banks = pQK 1 + pV 1 + pAV 1 + pS 3 singles (+2 spare);
    tail swaps pQK/pV/spare for two [128,1024] pair slots.  start=True
    clears has_written for the WHOLE bank: one start per bank
    lifetime; AV accumulation chains stay i-major.
  - HAM: only bf16 matmuls register as PE-busy; a short bf16 dummy
    chain + 1-element exp pre-warm the clock and pre-load the ACT exp
    table.  Long filler chains measurably hurt (the kernel is not
    PE-clock-bound).  Run-to-run variance from HAM phase is several us.
"""

from contextlib import ExitStack

import numpy as np

import concourse.bass as bass
import concourse.mybir as mybir
from concourse.tile import TileContext, add_dep_helper
from concourse.masks import make_upper_triangular
from concourse.bass_utils import run_bass_kernel_spmd

B, T, E, H = 8, 2048, 1024, 64
NT = T // 128   # 16 t-chunks
NE = E // 128   # 8 e-chunks
NG = 4          # t-groups of 4 chunks / 512 cols
F32 = mybir.dt.float32
F32R = mybir.dt.float32r
BF16 = mybir.dt.bfloat16
SCALE = float(H) ** -0.5
EXP = mybir.ActivationFunctionType.Exp


def _split_excess_waits(nc: bass.Bass, cap: int = 1) -> int:
    n_split = 0
    for f in nc.m.functions:
        for bb in f.blocks:
            insts = list(bb.instructions)
            out = []
            dirty = False
            for inst in insts:
                si = inst.sync_info
                waits = list(si.on_wait) if si and si.on_wait else []
                if len(waits) > cap:
                    si.on_wait = waits[:cap]
                    for w in waits[cap:]:
                        nop = mybir.InstNoOp(
                            name=f"I-waitsplit-{n_split}", ins=[], outs=[]
                        )
                        nop.engine = inst.engine
                        nop.sync_info = mybir.SyncInfo(on_wait=[w], on_update=[])
                        out.append(nop)
                        n_split += 1
                    dirty = True
                out.append(inst)
            if dirty:
                bb.instructions = out
    return n_split


def build_nc(split_waits: bool = True) -> bass.Bass:
    nc = bass.Bass()
    x = nc.dram_tensor("x", [T, E], BF16, kind="ExternalInput")
    wqk_d = nc.dram_tensor("Wqk", [128, NE * 128], BF16, kind="ExternalInput")
    wvb_d = nc.dram_tensor("Wvb", [128, NE * H], BF16, kind="ExternalInput")
    out = nc.dram_tensor("out", [T, H], F32, kind="ExternalOutput")
    x_ap, out_ap = x.ap(), out.ap()

    with TileContext(nc) as tc:
        with (
            tc.tile_pool(name="const", bufs=1) as cpool,
            tc.tile_pool(name="wts", bufs=1) as wpool,
            tc.tile_pool(name="xtg", bufs=4) as xtpool,
            tc.tile_pool(name="qkv", bufs=1) as qkvpool,
            tc.tile_pool(name="wTp", bufs=8) as wtpool,
            tc.tile_pool(name="fin", bufs=8) as finpool,
            tc.tile_pool(name="pS", bufs=3, space="PSUM") as pS,
            tc.tile_pool(name="pAV", bufs=1, space="PSUM") as pAV,
        ):
            ctx_stack = ExitStack()
            ingest_pools = ExitStack()
            pQK = ingest_pools.enter_context(
                tc.tile_pool(name="pQK", bufs=1, space="PSUM")
            )
            pV = ingest_pools.enter_context(
                tc.tile_pool(name="pV", bufs=1, space="PSUM")
            )
            # ---- PE warm-up + ACT exp-table preload ----
            wdum = cpool.tile([128, 512], BF16, tag="wdum")
            nc.gpsimd.iota(wdum[:], pattern=[[1, 512]], channel_multiplier=7,
                           allow_small_or_imprecise_dtypes=True)
            tabscr = cpool.tile([128, 8], BF16, tag="tabscr")
            nc.vector.memset(tabscr[:], 0.0)
            nc.scalar.activation(tabscr[:, 0:1], tabscr[:, 4:5], EXP,
                                 scale=1.0)
            psd = pS.tile([128, 512], F32, tag="ps", name="psdummy")
            for k in range(11):
                nc.tensor.matmul(psd[:, 0:512], wdum[:, 0:128], wdum[:],
                                 start=True, stop=True)
            # ---- x + weights on the two HWDGE rings.  Group-0 chunks
            # first as half-chunks striped across both rings; weights
            # follow; the rest full-chunk DMAs alternating rings. ----
            # x rides the SWDGE (gpsimd) queue with an in-flight
            # f32->bf16 cast: (a) bf16 transposes run ~3x faster than the
            # fp32-HIGH path on the PE, (b) SWDGE is a third descriptor
            # path, so no DMA instruction ever head-of-line-blocks the
            # sync/scalar compute queues, (c) SBUF writes halve.  Tiles
            # are fully resident (no pool-WAR waits on any x DMA).
            # Group-0 chunks go individually for a fast start; later
            # groups as 2 MB quad-chunk DMAs for line-rate.
            # constants first: gpsimd ops must not queue behind DMAs
            tri = cpool.tile([128, 128], BF16, tag="tri")
            make_upper_triangular(nc, tri[:], val=1.0, diag=True)
            # ALL DMAs ride the sync ring in strict FIFO order (weights,
            # then the 16 chunk transposes, then outputs): the xbar
            # transpose guard serializes transposes against every other
            # in-flight DMA, so cross-ring traffic ping-pongs and
            # strangles the stream.  Weights arrive pre-packed bf16 from
            # the host (no on-chip prep).  The scalar (ACT) queue carries
            # zero DMA instructions.
            wqk = wpool.tile([128, NE * 128], BF16, tag="wqk")
            wvb = wpool.tile([128, NE * H], BF16, tag="wvb")

            def weight_dmas():
                # emitted AFTER the first four chunk transposes: the ring
                # opens with t0 ~2us earlier, and the weights still land
                # well before the first v/qk matmuls need them
                nc.sync.dma_start(wqk[:], wqk_d.ap())
                nc.sync.dma_start(wvb[:], wvb_d.ap())

            # ---- persistent tiles ----
            qTt = qkvpool.tile([64, T], BF16, tag="qTt")
            kTt = qkvpool.tile([64, T], BF16, tag="kTt")
            v1 = qkvpool.tile([128, NT * 65], BF16, tag="v1")
            nc.vector.memset(
                v1[:].rearrange("p (i c) -> p i c", c=65)[:, :, 64:65], 1.0
            )
            xtg = [
                xtpool.tile([128, NE * 512], BF16, tag="xtg", name=f"xtg{g}")
                for g in range(NG)
            ]
            # paired wT storage: wTp[m] lane0 = j=2m, lane1 = j=2m+1,
            # interleaved per 512-col t-group block
            wTp = [
                wtpool.tile([128, 2 * T], BF16, tag="wTp", name=f"wTp{m}")
                for m in range(NT // 2)
            ]

            def wT_ap(j, lo, hi):
                """Columns [lo:hi) (absolute t) of wT[j] in paired layout.
                Must not cross a 512-col group boundary."""
                g = lo // 512
                l0, l1 = lo - 512 * g, hi - 512 * g
                base = 1024 * g + 512 * (j % 2)
                return wTp[j // 2][:, base + l0 : base + l1]

            stage = [
                finpool.tile([128, 256], F32, tag="stage", bufs=2, name=f"st{g}")
                for g in range(NG)
            ]

            # ---------- emission unit generators ----------
            def t_units(g):
                """Transpose the 4 x-chunks of group g into xtg[g] via the
                DMA xbar, straight from DRAM (contiguous chunk-major
                destination slabs: a strided dst corrupts on HW)."""
                for c in range(4):
                    i = 4 * g + c
                    def unit(i=i, c=c):
                        dst = (
                            xtg[i // 4][:, 1024 * c : 1024 * c + 1024]
                            .rearrange("p (j t) -> p j t", t=128)
                        )
                        nc.sync.dma_start_transpose(
                            dst, x_ap[128 * i : 128 * i + 128, :]
                        )
                    yield unit

            def v_units(g):
                """v-projection for group g into the hi bank of the group's
                pQKV pair tile; unit c needs only chunk c's columns."""
                def v_unit(c):
                    def unit():
                        if c == 0:
                            v_units.pv = pV.tile([128, 512], F32, tag="pv",
                                                 name=f"pv{g}")
                        pv = v_units.pv
                        for j in range(NE):
                            nc.tensor.matmul(
                                pv[:, 64 * c : 64 * c + 64],
                                xtg[g][:, 1024 * c + 128 * j : 1024 * c + 128 * j + 128],
                                wvb[:, 64 * j : 64 * j + 64],
                                start=(j == 0),
                                stop=(j == NE - 1),
                            )
                        if c == 3:
                            i0 = 4 * g
                            dst = (
                                v1[:]
                                .rearrange("p (i c) -> p i c", c=65)[
                                    :, i0 : i0 + 4, 0:64
                                ]
                            )
                            nc.vector.tensor_copy(
                                dst,
                                v_units.pv[:, 0:256].rearrange(
                                    "p (i c) -> p i c", c=64
                                ),
                            )
                    return unit
                for c in range(4):
                    yield v_unit(c)

            def qk_units(g):
                """qk-projection for group g into the lo bank of the
                group's pQKV tile (needs the whole group transposed)."""
                def qk_unit(jpair):
                    def unit():
                        if jpair == 0:
                            qk_units.pqk = pQK.tile(
                                [128, 512], F32, tag="pqk", name=f"pqk{g}"
                            )
                        pqk = qk_units.pqk[:]
                        xv = xtg[g][:].rearrange(
                            "p (c j t) -> p j c t", c=4, t=128
                        )
                        for j in (2 * jpair, 2 * jpair + 1):
                            nc.tensor.matmul(
                                pqk[:].rearrange("p (c t) -> p c t", t=128),
                                wqk[:, 128 * j : 128 * j + 128],
                                xv[:, j],
                                start=(j == 0),
                                stop=(j == NE - 1),
                            )
                        if jpair == 3:
                            nc.vector.tensor_copy(
                                qTt[:, 512 * g : 512 * g + 512], pqk[0:64, :]
                            )
                            nc.vector.tensor_copy(
                                kTt[:, 512 * g : 512 * g + 512], pqk[64:128, :]
                            )
                    return unit
                for jp in range(4):
                    yield qk_unit(jp)

            def qkvn_units(g):
                # qk first: it feeds scores(g) -> exp (the ACT critical
                # path); v(g) is not consumed until AV(g) a full loop later
                yield from qk_units(g)
                yield from v_units(g)

            def next_slot():
                s_units.idx = (s_units.idx + 1) % len(s_units.slots)
                return s_units.slots[s_units.idx]

            def next_pair_slot():
                s_units.pidx = (s_units.pidx + 1) % len(s_units.pair_slots)
                return s_units.pair_slots[s_units.pidx]

            def s_unit_single(j, g):
                """One 512-col score MM + exp for s-chunk j over t-block g
                (trim + tri-mask when diagonal)."""
                def unit():
                    pool, tag = next_slot()
                    off = max(0, 128 * j - 512 * g)
                    ps = pool.tile([128, 512], F32, tag=tag,
                                   name=f"pss{g}_{j}")
                    nc.tensor.matmul(
                        ps[:, off:512],
                        kTt[:, 128 * j : 128 * j + 128],
                        qTt[:, 512 * g + off : 512 * g + 512],
                        start=True,
                        stop=True,
                    )
                    nc.scalar.activation(
                        wT_ap(j, 512 * g + off, 512 * g + 512),
                        ps[:, off:512],
                        EXP,
                        scale=SCALE,
                    )
                    if j >= 4 * g:
                        nc.vector.tensor_mul(
                            wT_ap(j, 128 * j, 128 * j + 128),
                            wT_ap(j, 128 * j, 128 * j + 128),
                            tri[:],
                        )
                return unit

            def s_unit_pair(m, g):
                """Tail-only: two full score MMs into a [128,1024] pair
                tile, drained by ONE exp (halves the ACT fixed cost where
                ACT is the critical path)."""
                def unit():
                    pool, tag = next_pair_slot()
                    ps = pool.tile([128, 1024], F32, tag=tag,
                                   name=f"psp{g}_{m}")
                    for half in range(2):
                        j = 2 * m + half
                        nc.tensor.matmul(
                            ps[:, 512 * half : 512 * half + 512],
                            kTt[:, 128 * j : 128 * j + 128],
                            qTt[:, 512 * g : 512 * g + 512],
                            start=True,
                            stop=True,
                        )
                    nc.scalar.activation(
                        wTp[m][:, 1024 * g : 1024 * g + 1024],
                        ps[:],
                        EXP,
                        scale=SCALE,
                    )
                return unit

            def s_units(g):
                if s_units.pair_slots is not None:
                    for m in range(2 * g):
                        yield s_unit_pair(m, g)
                    for c in range(4):
                        yield s_unit_single(4 * g + c, g)
                else:
                    for j in range(4 * g + 4):
                        yield s_unit_single(j, g)

            def av_units(g):
                """AV accumulation for the 4 t-chunks of group g (i-major
                chains; dual psum banks in the tail)."""
                dual = len(av_units.pools) == 2
                if dual:
                    def alloc(g=g):
                        (pa, ta), (pb, tb) = av_units.pools
                        av_units.tiles[g] = (
                            pa.tile([128, 130], F32, tag=ta, name=f"pava{g}"),
                            pb.tile([128, 130], F32, tag=tb, name=f"pavb{g}"),
                        )
                    yield alloc
                    for p in range(2):
                        c0, c1 = 2 * p, 2 * p + 1
                        i0, i1 = 4 * g + c0, 4 * g + c1
                        js = list(range(i1 + 1))
                        batches = [js[k : k + 4] for k in range(0, len(js), 4)]
                        for batch in batches:
                            def unit(p=p, i0=i0, i1=i1, batch=batch, g=g):
                                pava, pavb = av_units.tiles[g]
                                off = 65 * p
                                for j in batch:
                                    if j <= i0:
                                        nc.tensor.matmul(
                                            pava[:, off : off + 65],
                                            wT_ap(j, 128 * i0, 128 * i0 + 128),
                                            v1[:, 65 * j : 65 * j + 65],
                                            start=(j == 0),
                                            stop=(j == i0),
                                        )
                                    nc.tensor.matmul(
                                        pavb[:, off : off + 65],
                                        wT_ap(j, 128 * i1, 128 * i1 + 128),
                                        v1[:, 65 * j : 65 * j + 65],
                                        start=(j == 0),
                                        stop=(j == i1),
                                    )
                            yield unit
                        # normalize + ship this chunk-pair immediately so
                        # the final output DMA covers only the last pair
                        def norm_out(p=p, c0=c0, c1=c1, g=g):
                            pava, pavb = av_units.tiles[g]
                            off = 65 * p
                            for pv_, c_ in ((pava, c0), (pavb, c1)):
                                rcp = finpool.tile([128, 1], F32, tag="rcp",
                                                   bufs=4)
                                nc.vector.reciprocal(
                                    rcp[:], pv_[:, off + 64 : off + 65]
                                )
                                nc.vector.tensor_scalar_mul(
                                    stage[g][:, 64 * c_ : 64 * c_ + 64],
                                    pv_[:, off : off + 64],
                                    rcp[:],
                                )
                            nc.sync.dma_start(
                                out_ap[512 * g + 256 * p : 512 * g + 256 * p + 256, :]
                                .rearrange("(c p2) h -> p2 c h", p2=128),
                                stage[g][:, 128 * p : 128 * p + 128]
                                .rearrange("p (c h) -> p c h", h=64),
                            )
                        yield norm_out
                else:
                    def alloc(g=g):
                        pa, ta = av_units.pools[0]
                        av_units.tiles[g] = pa.tile(
                            [128, 260], F32, tag=ta, name=f"pav{g}"
                        )
                    yield alloc
                    for c in range(4):
                        i = 4 * g + c
                        js = list(range(i + 1))
                        batches = [js[k : k + 4] for k in range(0, len(js), 4)]
                        for bi, batch in enumerate(batches):
                            def unit(i=i, c=c, batch=batch, g=g,
                                     last=(bi == len(batches) - 1)):
                                pav = av_units.tiles[g]
                                for j in batch:
                                    nc.tensor.matmul(
                                        pav[:, 65 * c : 65 * c + 65],
                                        wT_ap(j, 128 * i, 128 * i + 128),
                                        v1[:, 65 * j : 65 * j + 65],
                                        start=(j == 0),
                                        stop=(j == i),
                                    )
                                if last:
                                    # normalize this chunk as soon as its
                                    # chain completes (shortens the tail)
                                    rcp = finpool.tile([128, 1], F32,
                                                       tag="rcp", bufs=4)
                                    nc.vector.reciprocal(
                                        rcp[:],
                                        pav[:, 65 * c + 64 : 65 * c + 65],
                                    )
                                    nc.vector.tensor_scalar_mul(
                                        stage[g][:, 64 * c : 64 * c + 64],
                                        pav[:, 65 * c : 65 * c + 64],
                                        rcp[:],
                                    )
                            yield unit
                if not dual:
                    def dma_out_a(g=g):
                        nc.sync.dma_start(
                            out_ap[512 * g : 512 * g + 256, :]
                            .rearrange("(c p) h -> p c h", p=128),
                            stage[g][:, 0:128]
                            .rearrange("p (c h) -> p c h", h=64),
                        )
                    def dma_out_b(g=g):
                        nc.sync.dma_start(
                            out_ap[512 * g + 256 : 512 * g + 512, :]
                            .rearrange("(c p) h -> p c h", p=128),
                            stage[g][:, 128:256]
                            .rearrange("p (c h) -> p c h", h=64),
                        )
                    yield dma_out_a
                    yield dma_out_b

            def drain(*streams):
                streams = [s for s in streams if s is not None]
                while streams:
                    nxt = []
                    for s in streams:
                        u = next(s, None)
                        if u is not None:
                            u()
                            nxt.append(s)
                    streams = nxt

            s_units.slots = [(pS, "ps")]
            s_units.idx = 0
            s_units.pair_slots = None
            av_units.pools = [(pAV, "pav")]
            av_units.tiles = {}

            # ---------- prologue: group 0, v-proj fills chunk gaps ----------
            # weights FIRST: with all xtg slabs resident the transpose
            # stream runs gap-free (50ns inter-DMA), so the one mandatory
            # transpose<->plain-DMA guard boundary (~2.7us) must sit at
            # the stream START, not mid-stream where it costs ~6us.
            weight_dmas()
            tu = list(t_units(0))
            for c in range(4):
                tu[c]()
            drain(qk_units(0))
            drain(v_units(0))

            # ---------- steady loop ----------
            for g in range(NG):
                if g == NG - 1:
                    # transposes/projections done: free their psum banks and
                    # widen the scores rotation for the exp-heavy last group
                    ingest_pools.close()
                    pS3 = ctx_stack.enter_context(
                        tc.tile_pool(name="pS3", bufs=1, space="PSUM")
                    )
                    pS4 = ctx_stack.enter_context(
                        tc.tile_pool(name="pS4", bufs=1, space="PSUM")
                    )
                    s_units.pair_slots = [(pS3, "ps3"), (pS4, "ps4")]
                    s_units.pidx = 0
                # scores/AV first (they feed ACT, the back-half critical
                # resource); the arrival-gated ingest of g+1 afterwards so
                # its transposes never head-of-line-block score MMs.
                a = s_units(g)
                b = av_units(g - 1) if g >= 1 else None
                drain(a, b)
                if g + 1 < NG:
                    drain(t_units(g + 1))
                    drain(qkvn_units(g + 1))

            # ---------- epilogue: AV of the last group ----------
            drain(av_units(NG - 1))
            ctx_stack.close()

    if split_waits:
        _split_excess_waits(nc)
    return nc


_NC_CACHE = None


def _get_nc() -> bass.Bass:
    global _NC_CACHE
    if _NC_CACHE is None:
        _NC_CACHE = build_nc()
    return _NC_CACHE


def kernel(x, Wq, Wk, Wv, **run_kwargs):
    import ml_dtypes
    nc = _get_nc()
    x = np.ascontiguousarray(x).astype(ml_dtypes.bfloat16)
    # pre-pack the projection weights on the host exactly as the PE wants
    # them: wqk[p, 128j+c] = [Wq|Wk][128j+p, c], wvb[p, 64j+h] = Wv[128j+p, h]
    wq_r = Wq.reshape(NE, 128, H)
    wk_r = Wk.reshape(NE, 128, H)
    wv_r = Wv.reshape(NE, 128, H)
    wqk_np = np.concatenate([wq_r, wk_r], axis=2)      # [NE, 128, 128]
    wqk_np = wqk_np.transpose(1, 0, 2).reshape(128, NE * 128)
    wvb_np = wv_r.transpose(1, 0, 2).reshape(128, NE * H)
    wqk_np = np.ascontiguousarray(wqk_np).astype(ml_dtypes.bfloat16)
    wvb_np = np.ascontiguousarray(wvb_np).astype(ml_dtypes.bfloat16)
    in_maps = [
        {
            "x": np.ascontiguousarray(x[b]),
            "Wqk": wqk_np,
            "Wvb": wvb_np,
        }
        for b in range(B)
    ]
    res = run_bass_kernel_spmd(nc, in_maps, core_ids=list(range(B)), **run_kwargs)
    out = np.stack([res.results[b]["out"] for b in range(B)], axis=0)
    kernel.last_results = res
    return out
